# revision 18
# baseline (speedup 1.0000x reference)
"""Trainium2 Bass kernel for a pre-LN transformer block (nn_BaseBlock).

Reference computation (per batch b, fp32):
    h   = LN1(x); k,q,v = h@Wk+bk, h@Wq+bq, h@Wv+bv
    sim = (k @ q^T)/sqrt(E)  (causal tril mask), att = softmax(sim) @ v
    x2  = x + att
    h2  = LN2(x2)
    f   = gelu(gelu(gelu(h2@W1+b1)@W2a+b2a)@W2b+b2b)@W3 + b3
    out = x2 + f

Sharding over 8 cores: core c handles batch b=c//2, row half r=c%2
(interleaved i-tiles {2j+r} of that batch).  Every core computes
full-context q/v for its batch (cheap duplication) so a single SPMD
program runs on all cores; causality and row position enter only through
per-core data (mask, row order).  The context rows are permuted host-side
to [own rows | partner rows], so "own" is a uniform compile-time prefix:
no duplicate own-row LN, and the padded causal extent per own i-tile it is
exactly (it+1) own j-tiles + (it+1) partner j-tiles on every core.

All matmuls run in fp8(e4m3) with perf_mode=DoubleRow (2 k-tiles per MM;
measured ~1.7x over plain fp8 end-to-end on this hardware).  Weights are quantized host-side with per-matrix
power-of-2 scales; the dequant multiply rides the existing epilogue
(ACT scale operand / DVE scalar op) at zero extra cost.  Residual stream
stays fp32; LN/softmax statistics fp32.  Validated end-to-end fp8 error:
rel_fro ~2.5e-3 (tolerance 2e-2).

Score layout is transposed vs the math: simT[j,i] is computed directly
(lhsT=qT tile, rhs=kT own rows), so the softmax probabilities come out
j-major and feed the P@V matmul as the stationary operand without any
PE transposes.  Row sums come from a ones-column appended to v.
"""

import time

import numpy as np
import ml_dtypes

import concourse.bass as bass
import concourse.mybir as mybir
from concourse import bacc
import concourse.tile as tile
from concourse.bass_utils import run_bass_kernel_spmd

F32 = mybir.dt.float32
BF16 = mybir.dt.bfloat16
F8 = mybir.dt.float8e4
AF = mybir.ActivationFunctionType
ALU = mybir.AluOpType
AX = mybir.AxisListType
DR = mybir.MatmulPerfMode.DoubleRow
NP_F8 = ml_dtypes.float8_e4m3

EPS = 1e-5
N_CORES = 8
import os
USE_DR = os.environ.get("USE_DR", "1") == "1"  # DoubleRow fp8 matmuls
LN16 = float(np.log(16.0))  # softmax probs stored as 16*exp() in fp8


class Cfg:
    def __init__(self, E=1024, H=4096, T=2048, R=1024):
        self.E, self.H, self.T, self.R = E, H, T, R
        self.ET, self.HT, self.CT, self.RT = E // 128, H // 128, T // 128, R // 128
        self.scale = 1.0 / np.sqrt(E)


def _blocks(total, bs=512):
    return [(o, min(bs, total - o)) for o in range(0, total, bs)]


def build_program(cfg: Cfg, reps: int = 1):
    """Build the SPMD Bass program (one core's view).

    reps>1 wraps the body in repeated emission — used only for timing
    (amortizes the ~80ms axon dispatch round-trip over reps executions).
    """
    E, H, T, R = cfg.E, cfg.H, cfg.T, cfg.R
    ET, HT, CT, RT = cfg.ET, cfg.HT, cfg.CT, cfg.RT
    EB = _blocks(E)

    nc = bacc.Bacc("TRN2", target_bir_lowering=False, debug=False,
                   num_devices=N_CORES)

    # ---- DRAM I/O ----
    # x_b rows are permuted host-side to [own rows | partner rows].
    x_b = nc.dram_tensor("x_b", [T, E], F32, kind="ExternalInput")
    maskT = nc.dram_tensor("maskT", [T, R], F32, kind="ExternalInput")
    wqt = nc.dram_tensor("wqt", [ET, 128, ET, 128], F8, kind="ExternalInput")
    wkt = nc.dram_tensor("wkt", [ET, 128, ET, 128], F8, kind="ExternalInput")
    wv = nc.dram_tensor("wv", [E, E], F8, kind="ExternalInput")
    bq = nc.dram_tensor("bq", [E], F32, kind="ExternalInput")
    bk = nc.dram_tensor("bk", [E], F32, kind="ExternalInput")
    bv = nc.dram_tensor("bv", [E], F32, kind="ExternalInput")
    w1t = nc.dram_tensor("w1t", [HT, 128, ET, 128], F8, kind="ExternalInput")
    w2at = nc.dram_tensor("w2at", [HT, 128, HT, 128], F8, kind="ExternalInput")
    w2bt = nc.dram_tensor("w2bt", [HT, 128, HT, 128], F8, kind="ExternalInput")
    b1 = nc.dram_tensor("b1", [H], F32, kind="ExternalInput")
    b2a = nc.dram_tensor("b2a", [H], F32, kind="ExternalInput")
    b2b = nc.dram_tensor("b2b", [H], F32, kind="ExternalInput")
    w3t = nc.dram_tensor("w3t", [len(EB), HT, 128, EB[0][1]], F8,
                         kind="ExternalInput")
    b3 = nc.dram_tensor("b3", [E], F32, kind="ExternalInput")
    dqs = nc.dram_tensor("dqs", [128, 8], F32, kind="ExternalInput")
    ident_in = nc.dram_tensor("ident_in", [128, 128], BF16, kind="ExternalInput")
    out = nc.dram_tensor("out", [R, E], F32, kind="ExternalOutput")

    d = locals()
    with tile.TileContext(nc) as tc:
        for _ in range(reps):
            _emit(tc, cfg, d)
    nc.compile()
    return nc


def _ln_tile(nc, pools, x_ap, out_bf, eps_t, E):
    """Plain normalization of one [128, E] token tile: out_bf = (x-mu)*rstd.

    The LN affine (w, b) is folded into the downstream weight matrices on the
    host, so only the statistics part runs on-device.  Stats come from a
    single DVE bn_stats/bn_aggr pass (mean + var in one read); x_ap is not
    modified.
    """
    p = pools["ln_stats"]
    nchunk = (E + 511) // 512
    st = p.tile([128, nchunk, 6], F32, tag="ln_st")
    for c in range(nchunk):
        nc.vector.bn_stats(out=st[:, c, :], in_=x_ap[:, c * 512:(c + 1) * 512])
    agg = p.tile([128, 2], F32, tag="ln_agg")  # [mean, var]
    nc.vector.bn_aggr(out=agg[:], in_=st[:])
    sd = p.tile([128, 1], F32, tag="ln_sd")
    nc.scalar.activation(out=sd[:], in_=agg[:, 1:2], func=AF.Sqrt,
                         bias=eps_t[:], scale=1.0)
    rinv = p.tile([128, 1], F32, tag="ln_rinv")
    nc.vector.reciprocal(out=rinv[:], in_=sd[:])
    nc.vector.tensor_scalar(out=out_bf, in0=x_ap, scalar1=agg[:, 0:1],
                            scalar2=rinv[:], op0=ALU.subtract, op1=ALU.mult)


def _emit(tc, cfg, d):
    nc = tc.nc
    E, H, T, R = cfg.E, cfg.H, cfg.T, cfg.R
    ET, HT, CT, RT = cfg.ET, cfg.HT, cfg.CT, cfg.RT
    EB, TB, RB = _blocks(E), _blocks(T), _blocks(R)
    EP, HP = ET // 2, HT // 2  # k-tile pair counts (DoubleRow)
    VW = E + 16  # vtm width: E v-columns + ones column at E
    x_b, maskT, out = d["x_b"], d["maskT"], d["out"]

    import contextlib
    ctx = contextlib.ExitStack()
    with ctx:
        # ---------- constant / persistent pools ----------
        consts = ctx.enter_context(tc.tile_pool(name="consts", bufs=1))
        mm_ps = ctx.enter_context(tc.tile_pool(name="mm_ps", bufs=4, space="PSUM"))
        tr_ps = ctx.enter_context(tc.tile_pool(name="tr_ps", bufs=2, space="PSUM"))
        ls_ps = ctx.enter_context(tc.tile_pool(name="ls_ps", bufs=2, space="PSUM"))
        pools = {"ln_stats": ctx.enter_context(tc.tile_pool(name="ln_stats", bufs=3))}

        eps_t = consts.tile([128, 1], F32)
        nc.vector.memset(eps_t[:], EPS)
        ln16_t = consts.tile([128, 1], F32, tag="ln16")
        nc.vector.memset(ln16_t[:], LN16)
        ident = consts.tile([128, 128], BF16)
        nc.sync.dma_start(out=ident[:], in_=d["ident_in"].ap())
        dq = consts.tile([128, 8], F32)
        nc.sync.dma_start(out=dq[:], in_=d["dqs"].ap())
        # dequant scale columns: 0=q 1=k 2=v 3=w1 4=w2a 5=w2b 6=w3

        def bcast(name, dr, dtype=F32, width=None):
            w = width or dr.shape[0]
            t = consts.tile([128, w], dtype, tag=name)
            src = dr.ap()
            src_b = bass.AP(tensor=src.tensor, offset=src.offset,
                            ap=[[0, 128]] + list(src.ap))
            eng = nc.gpsimd if dtype != dr.dtype else nc.sync
            eng.dma_start(out=t[:], in_=src_b)
            return t

        def cols(name, dr, nt):
            t = consts.tile([128, nt], F32, tag=name)
            nc.sync.dma_start(out=t[:], in_=dr.ap().rearrange("(t p) -> p t", p=128))
            return t

        h2T_pool = ctx.enter_context(tc.tile_pool(name="h2T_pool", bufs=1))
        h2T = h2T_pool.tile([128, ET, R], F8, tag="h2T")
        x2_pool = ctx.enter_context(tc.tile_pool(name="x2", bufs=1))
        x2 = x2_pool.tile([128, RT, E], F32)  # residual stream (own rows), fp32

        # ================= attention block =================
        with tc.tile_pool(name="attn_big", bufs=1) as abig:
            qT = abig.tile([128, ET, T], F8, tag="qT")
            kT = abig.tile([128, ET, R], F8, tag="kT")
            # vtm/P j-dims split [jt, s]: s=0 own block, s=1 partner block,
            # so an (own jt, partner jt) DoubleRow pair is a contiguous slice.
            vtm = abig.tile([128, RT, 2, VW], F8, tag="vtm")  # token-major v
            P = abig.tile([128, RT, 2, R], F8, tag="P")  # probs (j-major)
            nc.vector.memset(vtm[:, :, :, E:E + 16], 1.0)  # ones col (row sums)

            with tc.tile_pool(name="hT_pool", bufs=1) as hp:
                hT = hp.tile([128, ET, T], F8, tag="hT")

                # ---- phase 1 + 2: ctx-LN (own tiles first) with v chasing;
                # k matmuls slot in once the own half of hT is ready ----
                with tc.tile_pool(name="ln_work", bufs=2) as lw, \
                     tc.tile_pool(name="ln_out", bufs=3) as lo:
                    def ln_transpose(t):
                        xt = lw.tile([128, E], F32, tag="xt")
                        nc.sync.dma_start(out=xt[:],
                                          in_=x_b.ap()[t * 128:(t + 1) * 128, :])
                        hbf = lo.tile([128, E], BF16, tag="hbf")
                        _ln_tile(nc, pools, xt[:], hbf[:], eps_t, E)
                        for et in range(ET):
                            tp = tr_ps.tile([128, 128], BF16, tag="tr")
                            nc.tensor.transpose(
                                tp[:], hbf[:, et * 128:(et + 1) * 128], ident[:])
                            nc.vector.tensor_copy(
                                out=hT[:, et, t * 128:(t + 1) * 128], in_=tp[:])

                    bq_c = cols("bq", d["bq"], ET)
                    bk_c = cols("bk", d["bk"], ET)
                    wv_ctx = tc.tile_pool(name="wv_pool", bufs=1)
                    wvp = wv_ctx.__enter__()
                    wv_sb = wvp.tile([128, ET, E], F8)
                    bv_bc = bcast("bv", d["bv"])
                    wv_src = d["wv"].ap().rearrange("(kt p) e -> p kt e", p=128)
                    for kt in range(ET):
                        nc.sync.dma_start(out=wv_sb[:, kt, :], in_=wv_src[:, kt, :])

                    def v_mm(tt):
                        jt, s = tt % RT, tt // RT
                        for eo, en in EB:
                            ps = mm_ps.tile([128, 512], F32, tag="mm")
                            if USE_DR:
                                for kp in range(EP):
                                    nc.tensor.matmul(
                                        ps[:, :en],
                                        hT[:, 2 * kp:2 * kp + 2, tt * 128:(tt + 1) * 128],
                                        wv_sb[:, 2 * kp:2 * kp + 2, eo:eo + en],
                                        start=(kp == 0), stop=(kp == EP - 1),
                                        perf_mode=DR)
                            else:
                                for kt in range(ET):
                                    nc.tensor.matmul(
                                        ps[:, :en],
                                        hT[:, kt, tt * 128:(tt + 1) * 128],
                                        wv_sb[:, kt, eo:eo + en],
                                        start=(kt == 0), stop=(kt == ET - 1))
                            nc.vector.scalar_tensor_tensor(
                                out=vtm[:, jt, s, eo:eo + en], in0=ps[:, :en],
                                scalar=dq[:, 2:3], in1=bv_bc[:, eo:eo + en],
                                op0=ALU.mult, op1=ALU.add)

                    for tt in range(RT):  # own half of the context
                        ln_transpose(tt)
                        v_mm(tt)

                    # ---- k (own rows = first R columns of hT) ----
                    with tc.tile_pool(name="wk_stream", bufs=2) as wks:
                        for mt in range(ET):
                            wk_mt = wks.tile([128, ET, 128], F8, tag="wk_mt")
                            nc.sync.dma_start(out=wk_mt[:], in_=d["wkt"].ap()[mt])
                            for ro, rn in RB:
                                ps = mm_ps.tile([128, 512], F32, tag="mm")
                                if USE_DR:
                                    for kp in range(EP):
                                        nc.tensor.matmul(
                                            ps[:, :rn],
                                            wk_mt[:, 2 * kp:2 * kp + 2, :],
                                            hT[:, 2 * kp:2 * kp + 2, ro:ro + rn],
                                            start=(kp == 0), stop=(kp == EP - 1),
                                            perf_mode=DR)
                                else:
                                    for kt in range(ET):
                                        nc.tensor.matmul(
                                            ps[:, :rn], wk_mt[:, kt, :],
                                            hT[:, kt, ro:ro + rn],
                                            start=(kt == 0), stop=(kt == ET - 1))
                                nc.scalar.activation(
                                    out=kT[:, mt, ro:ro + rn], in_=ps[:, :rn],
                                    func=AF.Identity, bias=bk_c[:, mt:mt + 1],
                                    scale=dq[:, 1:2])

                    for tt in range(RT, CT):  # partner half of the context
                        ln_transpose(tt)
                        v_mm(tt)
                    wv_ctx.__exit__(None, None, None)

                # ---- q (full ctx), feature-major ----
                with tc.tile_pool(name="wq_stream", bufs=3) as wqs:
                    for mt in range(ET):
                        wq_mt = wqs.tile([128, ET, 128], F8, tag="wq_mt")
                        nc.sync.dma_start(out=wq_mt[:], in_=d["wqt"].ap()[mt])
                        for jo, jn in TB:
                            ps = mm_ps.tile([128, 512], F32, tag="mm")
                            if USE_DR:
                                for kp in range(EP):
                                    nc.tensor.matmul(
                                        ps[:, :jn], wq_mt[:, 2 * kp:2 * kp + 2, :],
                                        hT[:, 2 * kp:2 * kp + 2, jo:jo + jn],
                                        start=(kp == 0), stop=(kp == EP - 1),
                                        perf_mode=DR)
                            else:
                                for kt in range(ET):
                                    nc.tensor.matmul(
                                        ps[:, :jn], wq_mt[:, kt, :],
                                        hT[:, kt, jo:jo + jn],
                                        start=(kt == 0), stop=(kt == ET - 1))
                            nc.scalar.activation(
                                out=qT[:, mt, jo:jo + jn], in_=ps[:, :jn],
                                func=AF.Identity, bias=bq_c[:, mt:mt + 1],
                                scale=dq[:, 0:1])

            # ---- phase 3: attention scores (transposed) + AV ----
            # Scores per own-row block: simT[j, i] = sum_e q[j,e] k[i,e],
            # exp'd (x16, in-bias) straight into the j-major prob buffer P.
            # Padded causal extent per block: 4*(blk+1) j-tiles from the own
            # half plus the same count from the partner half.  The mask input
            # (data) provides exact causality incl. padding.
            with tc.tile_pool(name="at_mask", bufs=3) as mkp, \
                 tc.tile_pool(name="at_sim", bufs=2) as smp, \
                 tc.tile_pool(name="at_misc", bufs=3) as msc:
                for blk, (ro, rn) in enumerate(RB):
                    for s in range(2):
                        for jt in range(4 * (blk + 1)):
                            jc = (s * RT + jt) * 128  # column in permuted hT/qT
                            ps = mm_ps.tile([128, 512], F32, tag="mm")
                            if USE_DR:
                                for ep in range(EP):
                                    nc.tensor.matmul(
                                        ps[:, :rn],
                                        qT[:, 2 * ep:2 * ep + 2, jc:jc + 128],
                                        kT[:, 2 * ep:2 * ep + 2, ro:ro + rn],
                                        start=(ep == 0), stop=(ep == EP - 1),
                                        perf_mode=DR)
                            else:
                                for et in range(ET):
                                    nc.tensor.matmul(
                                        ps[:, :rn], qT[:, et, jc:jc + 128],
                                        kT[:, et, ro:ro + rn],
                                        start=(et == 0), stop=(et == ET - 1))
                            mk = mkp.tile([128, 512], F32, tag="mk")
                            nc.sync.dma_start(
                                out=mk[:, :rn],
                                in_=maskT.ap()[jc:jc + 128, ro:ro + rn])
                            sim = smp.tile([128, 512], F32, tag="sim")
                            nc.vector.tensor_tensor(out=sim[:, :rn], in0=ps[:, :rn],
                                                    in1=mk[:, :rn], op=ALU.add)
                            # No max-subtraction: |sim*scale| <= O(2) here, and
                            # the -1e30 mask underflows exp to exactly 0.  The
                            # x16 (ln16 bias) keeps small probs out of fp8
                            # subnormals; it cancels in the row-sum normalize.
                            nc.scalar.activation(out=P[:, jt, s, ro:ro + rn],
                                                 in_=sim[:, :rn], func=AF.Exp,
                                                 scale=float(cfg.scale),
                                                 bias=ln16_t[:])

                    # AV for the 4 own i-tiles of this block; DoubleRow pairs
                    # (own j-tile jp, partner j-tile jp) for jp <= it.
                    for itl in range(4):
                        it = blk * 4 + itl
                        npair = it + 1
                        lp = ls_ps.tile([128, 1], F32, tag="ls")
                        if USE_DR:
                            for jp in range(npair):
                                nc.tensor.matmul(
                                    lp[:], P[:, jp, :, it * 128:(it + 1) * 128],
                                    vtm[:, jp, :, E:E + 1],
                                    start=(jp == 0), stop=(jp == npair - 1),
                                    perf_mode=DR)
                        else:
                            for jp in range(npair):
                                for sj in range(2):
                                    nc.tensor.matmul(
                                        lp[:], P[:, jp, sj, it * 128:(it + 1) * 128],
                                        vtm[:, jp, sj, E:E + 1],
                                        start=(jp == 0 and sj == 0),
                                        stop=(jp == npair - 1 and sj == 1))
                        linv = msc.tile([128, 1], F32, tag="linv")
                        nc.vector.reciprocal(out=linv[:], in_=lp[:])
                        xo = msc.tile([128, E], F32, tag="xo", bufs=2)
                        nc.sync.dma_start(
                            out=xo[:], in_=x_b.ap()[it * 128:(it + 1) * 128, :])
                        for eo, en in EB:
                            ps = mm_ps.tile([128, 512], F32, tag="mm")
                            if USE_DR:
                                for jp in range(npair):
                                    nc.tensor.matmul(
                                        ps[:, :en],
                                        P[:, jp, :, it * 128:(it + 1) * 128],
                                        vtm[:, jp, :, eo:eo + en],
                                        start=(jp == 0), stop=(jp == npair - 1),
                                        perf_mode=DR)
                            else:
                                for jp in range(npair):
                                    for sj in range(2):
                                        nc.tensor.matmul(
                                            ps[:, :en],
                                            P[:, jp, sj, it * 128:(it + 1) * 128],
                                            vtm[:, jp, sj, eo:eo + en],
                                            start=(jp == 0 and sj == 0),
                                            stop=(jp == npair - 1 and sj == 1))
                            nc.vector.scalar_tensor_tensor(
                                out=x2[:, it, eo:eo + en], in0=ps[:, :en],
                                scalar=linv[:], in1=xo[:, eo:eo + en],
                                op0=ALU.mult, op1=ALU.add)

        # ---- phase 4 + g1: LN2 halves interleaved with W1 row-blocks so the
        # PE has matmul work during the (ACT/DVE-bound) second LN2 half ----
        b3_bc = bcast("b3", d["b3"])
        with tc.tile_pool(name="ln2_out", bufs=2) as l2o, \
             tc.tile_pool(name="gx", bufs=1) as gxp, \
             tc.tile_pool(name="mlp_ws", bufs=1) as ws:
            b1_c = cols("b1", d["b1"], HT)
            b2a_c = cols("b2a", d["b2a"], HT)
            b2b_c = cols("b2b", d["b2b"], HT)
            g1T = gxp.tile([128, HT, R], F8, tag="gx")
            for half, (ro, rn) in enumerate(RB):
                for rt in range(4 * half, 4 * half + 4):
                    h2bf = l2o.tile([128, E], BF16, tag="h2bf")
                    _ln_tile(nc, pools, x2[:, rt, :], h2bf[:], eps_t, E)
                    for et in range(ET):
                        tp = tr_ps.tile([128, 128], BF16, tag="tr")
                        nc.tensor.transpose(tp[:], h2bf[:, et * 128:(et + 1) * 128],
                                            ident[:])
                        nc.vector.tensor_copy(
                            out=h2T[:, et, rt * 128:(rt + 1) * 128], in_=tp[:])
                    nc.vector.tensor_tensor(out=x2[:, rt, :], in0=x2[:, rt, :],
                                            in1=b3_bc[:], op=ALU.add)
                # ---- g1 = gelu(h2 @ W1 + b1), feature-major (w1 streamed
                # twice — cheap) ----
                for mt in range(HT):
                    w1_mt = ws.tile([128, ET, 128], F8, tag="w1_mt", bufs=2)
                    nc.sync.dma_start(out=w1_mt[:], in_=d["w1t"].ap()[mt])
                    ps = mm_ps.tile([128, 512], F32, tag="mm")
                    if USE_DR:
                        for kp in range(EP):
                            nc.tensor.matmul(ps[:, :rn],
                                             w1_mt[:, 2 * kp:2 * kp + 2, :],
                                             h2T[:, 2 * kp:2 * kp + 2, ro:ro + rn],
                                             start=(kp == 0), stop=(kp == EP - 1),
                                             perf_mode=DR)
                    else:
                        for kt in range(ET):
                            nc.tensor.matmul(ps[:, :rn], w1_mt[:, kt, :],
                                             h2T[:, kt, ro:ro + rn],
                                             start=(kt == 0), stop=(kt == ET - 1))
                    nc.scalar.activation(out=g1T[:, mt, ro:ro + rn], in_=ps[:, :rn],
                                         func=AF.Gelu, bias=b1_c[:, mt:mt + 1],
                                         scale=dq[:, 3:4])

            # ---- g2 = gelu(g1 @ W2a + b2a); g3 = gelu(g2 @ W2b + b2b) ----
            # g3T reuses g1T's slot (same pool+tag); the weight-stream pool
            # spans all layers so prefetch crosses phase boundaries.
            with tc.tile_pool(name="g2", bufs=1) as g2p:
                g2T = g2p.tile([128, HT, R], F8, tag="g2")
                for mt in range(HT):
                    w2_mt = ws.tile([128, HT, 128], F8, tag="w2a_mt", bufs=2)
                    nc.sync.dma_start(out=w2_mt[:], in_=d["w2at"].ap()[mt])
                    for ro, rn in RB:
                        ps = mm_ps.tile([128, 512], F32, tag="mm")
                        if USE_DR:
                            for kp in range(HP):
                                nc.tensor.matmul(ps[:, :rn],
                                                 w2_mt[:, 2 * kp:2 * kp + 2, :],
                                                 g1T[:, 2 * kp:2 * kp + 2, ro:ro + rn],
                                                 start=(kp == 0), stop=(kp == HP - 1),
                                                 perf_mode=DR)
                        else:
                            for kt in range(HT):
                                nc.tensor.matmul(ps[:, :rn], w2_mt[:, kt, :],
                                                 g1T[:, kt, ro:ro + rn],
                                                 start=(kt == 0), stop=(kt == HT - 1))
                        nc.scalar.activation(out=g2T[:, mt, ro:ro + rn],
                                             in_=ps[:, :rn], func=AF.Gelu,
                                             bias=b2a_c[:, mt:mt + 1],
                                             scale=dq[:, 4:5])

                g3T = gxp.tile([128, HT, R], F8, tag="gx")
                for mt in range(HT):
                    w2_mt = ws.tile([128, HT, 128], F8, tag="w2b_mt", bufs=2)
                    nc.sync.dma_start(out=w2_mt[:], in_=d["w2bt"].ap()[mt])
                    for ro, rn in RB:
                        ps = mm_ps.tile([128, 512], F32, tag="mm")
                        if USE_DR:
                            for kp in range(HP):
                                nc.tensor.matmul(ps[:, :rn],
                                                 w2_mt[:, 2 * kp:2 * kp + 2, :],
                                                 g2T[:, 2 * kp:2 * kp + 2, ro:ro + rn],
                                                 start=(kp == 0), stop=(kp == HP - 1),
                                                 perf_mode=DR)
                        else:
                            for kt in range(HT):
                                nc.tensor.matmul(ps[:, :rn], w2_mt[:, kt, :],
                                                 g2T[:, kt, ro:ro + rn],
                                                 start=(kt == 0), stop=(kt == HT - 1))
                        nc.scalar.activation(out=g3T[:, mt, ro:ro + rn],
                                             in_=ps[:, :rn], func=AF.Gelu,
                                             bias=b2b_c[:, mt:mt + 1],
                                             scale=dq[:, 5:6])

            # ---- f = g3 @ W3 (+b3 already in x2); out = x2 + f ----
            with tc.tile_pool(name="w3_pool", bufs=2) as w3p, \
                 tc.tile_pool(name="out_pool", bufs=3) as op:
                for ebi, (eo, en) in enumerate(EB):
                    w3_sb = w3p.tile([128, HT, EB[0][1]], F8, tag="w3_sb")
                    # sub-chunked load: first matmuls start after 1/8 arrives
                    for kc in range(0, HT, max(1, HT // 8)):
                        kce = min(HT, kc + max(1, HT // 8))
                        nc.sync.dma_start(
                            out=w3_sb[:, kc:kce, :],
                            in_=d["w3t"].ap()[ebi, kc:kce].rearrange("kt p e -> p kt e"))
                    for tt in range(RT):
                        ps = mm_ps.tile([128, 512], F32, tag="mm")
                        if USE_DR:
                            for kp in range(HP):
                                nc.tensor.matmul(
                                    ps[:, :en],
                                    g3T[:, 2 * kp:2 * kp + 2, tt * 128:(tt + 1) * 128],
                                    w3_sb[:, 2 * kp:2 * kp + 2, :en],
                                    start=(kp == 0), stop=(kp == HP - 1),
                                    perf_mode=DR)
                        else:
                            for kt in range(HT):
                                nc.tensor.matmul(
                                    ps[:, :en],
                                    g3T[:, kt, tt * 128:(tt + 1) * 128],
                                    w3_sb[:, kt, :en],
                                    start=(kt == 0), stop=(kt == HT - 1))
                        ot = op.tile([128, EB[0][1]], F32, tag="ot")
                        nc.vector.scalar_tensor_tensor(
                            out=ot[:, :en], in0=ps[:, :en], scalar=dq[:, 6:7],
                            in1=x2[:, tt, eo:eo + en], op0=ALU.mult, op1=ALU.add)
                        nc.sync.dma_start(
                            out=out.ap()[tt * 128:(tt + 1) * 128, eo:eo + en],
                            in_=ot[:, :en])


# ---------------- host side ----------------

def _p2scale(w):
    """Power-of-2 scale putting max|w| safely under the TRN e4m3 max (240)."""
    m = float(np.abs(w).max())
    if m == 0.0:
        return 1.0
    return float(2.0 ** np.floor(np.log2(224.0 / m)))


def _q8(w, s):
    return np.clip(w * s, -240.0, 240.0).astype(NP_F8)


def _tile_lhs(w, s):
    """[K, M] -> [MT, 128, KT, 128] (per-m-tile contiguous lhsT blocks), fp8."""
    K, M = w.shape
    t = w.reshape(K // 128, 128, M // 128, 128).transpose(2, 1, 0, 3)
    return _q8(np.ascontiguousarray(t), s)


def own_rows(cfg: Cfg, r):
    """Row indices (within the batch) owned by core half r: i-tiles {2j+r}."""
    tiles = [2 * it + r for it in range(cfg.RT)]
    return np.concatenate([np.arange(t * 128, (t + 1) * 128) for t in tiles])


def prepare_core_inputs(inputs, cfg: Cfg, b, r):
    x = np.asarray(inputs["x"])
    rows = own_rows(cfg, r)
    perm = np.concatenate([rows, own_rows(cfg, 1 - r)])
    im = {
        "x_b": np.ascontiguousarray(x[b][perm]),
        "b2a": np.asarray(inputs["b2a"]), "b2b": np.asarray(inputs["b2b"]),
        "b3": np.asarray(inputs["b3"]),
        "ident_in": np.eye(128, dtype=ml_dtypes.bfloat16),
    }
    im["maskT"] = np.where(perm[:, None] <= rows[None, :], 0.0,
                           -1e30).astype(np.float32)
    return im


def prepare_shared_weights(inputs, cfg: Cfg):
    """Quantize/tile weights to fp8; fold the LN affines into the downstream
    matmuls: (n*w + b) @ W + c  ==  n @ (diag(w) W) + (b @ W + c)."""
    E, H = cfg.E, cfg.H
    ln1_w, ln1_b = np.asarray(inputs["ln1_w"]), np.asarray(inputs["ln1_b"])
    ln2_w, ln2_b = np.asarray(inputs["ln2_w"]), np.asarray(inputs["ln2_b"])
    Wq, Wk, Wv = (np.asarray(inputs[k]) for k in ("Wq", "Wk", "Wv"))
    W1 = np.asarray(inputs["W1"])
    W2a, W2b, W3 = (np.asarray(inputs[k]) for k in ("W2a", "W2b", "W3"))
    wq_e = ln1_w[:, None] * Wq
    wk_e = ln1_w[:, None] * Wk
    wv_e = ln1_w[:, None] * Wv
    bq_e = ln1_b @ Wq + np.asarray(inputs["bq"])
    bk_e = ln1_b @ Wk + np.asarray(inputs["bk"])
    bv_e = ln1_b @ Wv + np.asarray(inputs["bv"])
    w1_e = ln2_w[:, None] * W1
    b1_e = ln2_b @ W1 + np.asarray(inputs["b1"])

    sq, sk, sv = _p2scale(wq_e), _p2scale(wk_e), _p2scale(wv_e)
    s1, s2a, s2b, s3 = (_p2scale(w) for w in (w1_e, W2a, W2b, W3))

    eb = _blocks(E)
    w3r = np.ascontiguousarray(
        W3.reshape(H // 128, 128, len(eb), eb[0][1]).transpose(2, 0, 1, 3))
    dqs = np.tile(np.array([1 / sq, 1 / sk, 1 / sv, 1 / s1, 1 / s2a, 1 / s2b,
                            1 / s3, 0.0], np.float32), (128, 1))
    return {
        "wqt": _tile_lhs(wq_e, sq),
        "wkt": _tile_lhs(wk_e, sk),
        "wv": _q8(wv_e, sv),
        "bq": bq_e.astype(np.float32), "bk": bk_e.astype(np.float32),
        "bv": bv_e.astype(np.float32),
        "w1t": _tile_lhs(w1_e, s1),
        "b1": b1_e.astype(np.float32),
        "w2at": _tile_lhs(W2a, s2a),
        "w2bt": _tile_lhs(W2b, s2b),
        "w3t": _q8(w3r, s3),
        "dqs": dqs,
    }


_PROGRAM_CACHE = {}


def get_program(cfg: Cfg, reps: int = 1):
    key = (cfg.E, cfg.H, cfg.T, cfg.R, reps)
    if key not in _PROGRAM_CACHE:
        _PROGRAM_CACHE[key] = build_program(cfg, reps=reps)
    return _PROGRAM_CACHE[key]


def _make_in_maps(inputs, cfg: Cfg):
    shared = prepare_shared_weights(inputs, cfg)
    in_maps = []
    for c in range(N_CORES):
        b, r = c // 2, c % 2
        im = prepare_core_inputs(inputs, cfg, b, r)
        im.update(shared)
        in_maps.append(im)
    return in_maps


def run(inputs, cfg: Cfg, trace=False):
    nc = get_program(cfg)
    in_maps = _make_in_maps(inputs, cfg)
    res = run_bass_kernel_spmd(nc, in_maps, core_ids=list(range(N_CORES)),
                               trace=trace)
    B = np.asarray(inputs["x"]).shape[0]
    T_full = np.asarray(inputs["x"]).shape[1]
    outp = np.empty((B, T_full, cfg.E), np.float32)
    for c in range(N_CORES):
        b, r = c // 2, c % 2
        outp[b][own_rows(cfg, r)] = res.results[c]["out"]
    return outp, res


def _build_sharded_exec(nc, in_maps):
    """Mirror bass2jax.run_bass_via_pjrt but return a reusable timed runner."""
    import jax
    from jax.sharding import Mesh, PartitionSpec, NamedSharding
    from jax.experimental.shard_map import shard_map
    import concourse.mybir as mb
    from concourse import bass2jax

    bass2jax.install_neuronx_cc_hook()
    n_cores = len(in_maps)
    partition_name = (nc.partition_id_tensor.name
                      if nc.partition_id_tensor is not None else None)
    in_names, out_names, out_avals, zero_outs = [], [], [], []
    for alloc in nc.m.functions[0].allocations:
        if not isinstance(alloc, mb.MemoryLocationSet):
            continue
        name = alloc.memorylocations[0].name
        if alloc.kind == "ExternalInput":
            if name != partition_name:
                in_names.append(name)
        elif alloc.kind == "ExternalOutput":
            out_names.append(name)
            shape = tuple(alloc.tensor_shape)
            dtype = mb.dt.np(alloc.dtype)
            out_avals.append(jax.core.ShapedArray(shape, dtype))
            zero_outs.append(np.zeros(shape, dtype))
    n_params = len(in_names)
    n_outs = len(out_avals)
    all_names = in_names + out_names
    if partition_name is not None:
        all_names = all_names + [partition_name]

    def _call_once(params, zouts):
        operands = list(params) + list(zouts)
        if partition_name is not None:
            operands.append(bass2jax.partition_id_tensor())
        outs = bass2jax._bass_exec_p.bind(
            *operands,
            out_avals=tuple(out_avals),
            in_names=tuple(all_names),
            out_names=tuple(out_names),
            lowering_input_output_aliases=(),
            sim_require_finite=True,
            sim_require_nnan=True,
            nc=nc,
        )
        return tuple(outs)

    def make_body(chain):
        def _body(*args):
            params = args[:n_params]
            outs = args[n_params:]
            for _ in range(chain):
                outs = _call_once(params, outs)
            return tuple(outs)
        return _body

    devices = jax.devices()[:n_cores]
    mesh = Mesh(np.asarray(devices), ("core",))
    in_specs = (PartitionSpec("core"),) * (n_params + n_outs)
    out_specs = (PartitionSpec("core"),) * n_outs
    donate = tuple(range(n_params, n_params + n_outs))

    def make_sharded(chain):
        return jax.jit(
            shard_map(make_body(chain), mesh=mesh, in_specs=in_specs,
                      out_specs=out_specs, check_rep=False),
            donate_argnums=donate, keep_unused=True)

    sharded = make_sharded(1)

    sh = NamedSharding(mesh, PartitionSpec("core"))
    concat_in = [
        jax.device_put(
            np.concatenate([np.asarray(in_maps[c][nm]) for c in range(n_cores)],
                           axis=0), sh)
        for nm in in_names
    ]

    def make_zeros():
        return [jax.device_put(
            np.zeros((n_cores * z.shape[0], *z.shape[1:]), z.dtype), sh)
            for z in zero_outs]

    _jit_cache = {1: sharded}

    def runner(chain=1, nruns=1):
        if chain not in _jit_cache:
            _jit_cache[chain] = make_sharded(chain)
        fn = _jit_cache[chain]
        all_zs = [make_zeros() for _ in range(nruns)]
        for zs in all_zs:
            for z in zs:
                z.block_until_ready()
        t0 = time.perf_counter()
        outs_l = [fn(*concat_in, *zs) for zs in all_zs]
        for outs in outs_l:
            for o in outs:
                o.block_until_ready()
        return time.perf_counter() - t0, outs_l[-1]

    return runner, out_names


def time_exec(inputs, cfg: Cfg, iters=8, reps=3):
    """Per-execution device time via a NEFF containing `reps` unrolled copies
    of the kernel body, differenced against reps=1 to cancel the ~80 ms axon
    dispatch round-trip.  Returns (per_exec_estimates, t1_list, tk_list)."""
    in_maps = _make_in_maps(inputs, cfg)
    r1, _ = _build_sharded_exec(get_program(cfg, reps=1), in_maps)
    rk, _ = _build_sharded_exec(get_program(cfg, reps=reps), in_maps)
    r1(); rk()  # warm both
    t1s, tks = [], []
    for _ in range(iters):
        t1, _ = r1()
        tk, _ = rk()
        t1s.append(t1)
        tks.append(tk)
    med = (np.median(tks) - np.median(t1s)) / (reps - 1)
    return med, t1s, tks


def kernel(**inputs) -> np.ndarray:
    cfg = Cfg(E=1024, H=4096, T=2048, R=1024)
    outp, _ = run(inputs, cfg)
    return outp
